# revision 27
# baseline (speedup 1.0000x reference)
"""GRU trajectory head on 8 Trainium2 NeuronCores.

Problem: B=4096, D=1024, H=1024, T=50, FREE_RATIO=0.0 (teacher forcing
always wins, so the GRU input at step t is teacher_deltas[:, t-1] and
step 0 input is zeros — the recurrence only flows through h).

Sharding: data-parallel over batch, 512 rows per core; weights replicated.

Per-core layout (feature dim on partitions, batch on free dim):
  h16[j]      [128, 512] fp16, j=0..7  (hidden state, ping-pong buffered)
  gh psum     [128, 512] accumulates  w_hhT k-tiles (8 fp16 matmuls)
                                      + w_ihT K=2 matmul (teacher input)
  gates       ACT sigmoid/tanh with per-partition bias, DVE for products
  delta       [2, 512] psum via w_out k-tile matmuls on the new h
"""

import sys

sys.path.insert(0, "/opt/trn_rl_repo")

import numpy as np

B, D, H, T = 4096, 1024, 1024, 50
NCORES = 8
BS = B // NCORES  # 512 batch rows per core
KT = H // 128  # 8 k-tiles over the hidden dim
MT = H // 128  # 8 m-tiles per gate

_PROGRAM = None


def _split_waits(nc, limit=1):
    # This walrus build allows few sync-wait commands per instruction
    # (1 on ACT); hoist excess waits onto same-engine NoOps just before.
    import concourse.mybir as mybir

    for fn in nc.m.functions:
        for bb in list(fn.blocks):
            new_list = []
            changed = False
            for inst in bb.instructions:
                si = getattr(inst, "sync_info", None)
                waits = list(si.on_wait) if si is not None and si.on_wait else []
                if len(waits) > limit:
                    changed = True
                    excess, keep = waits[:-limit], waits[-limit:]
                    for j in range(0, len(excess), limit):
                        nop = mybir.InstNoOp(
                            name=nc.get_next_instruction_name(),
                            engine=inst.engine,
                            bass_nofuse=True,
                            sync_info=mybir.SyncInfo(
                                on_wait=excess[j : j + limit], on_update=[]
                            ),
                        )
                        nc.register_instruction(nop)
                        new_list.append(nop)
                    inst.sync_info = mybir.SyncInfo(on_wait=keep, on_update=si.on_update)
                new_list.append(inst)
            if changed:
                bb.instructions = new_list


def _build(steps=T):
    import concourse.bass as bass
    import concourse.mybir as mybir
    from concourse.tile import TileContext

    F32 = mybir.dt.float32
    F16 = mybir.dt.float16
    AF = mybir.ActivationFunctionType
    ALU = mybir.AluOpType

    nc = bass.Bass()

    evT = nc.declare_dram_parameter("evT", [H, BS], F16, isOutput=False)
    teachT = nc.declare_dram_parameter("teachT", [2, steps * BS], F16, isOutput=False)
    w_initT = nc.declare_dram_parameter("w_initT", [D, H], F16, isOutput=False)
    w_hhT = nc.declare_dram_parameter(
        "w_hhT", [MT * KT * 128, 384], F16, isOutput=False
    )
    w_ihT = nc.declare_dram_parameter("w_ihT", [128, 3 * H], F16, isOutput=False)
    woutKM = nc.declare_dram_parameter("woutKM", [128, 128 * KT], F16, isOutput=False)
    b_init_t = nc.declare_dram_parameter("b_init_t", [128, MT], F32, isOutput=False)
    b_r_t = nc.declare_dram_parameter("b_r_t", [128, MT], F32, isOutput=False)
    b_z_t = nc.declare_dram_parameter("b_z_t", [128, MT], F32, isOutput=False)
    b_hn_t = nc.declare_dram_parameter("b_hn_t", [128, MT], F32, isOutput=False)
    b_in_t = nc.declare_dram_parameter("b_in_t", [128, MT], F32, isOutput=False)
    b_out_c = nc.declare_dram_parameter("b_out_c", [2, 1], F32, isOutput=False)
    deltasT = nc.declare_dram_parameter("deltasT", [steps, 2, BS], F32, isOutput=True)
    posT = nc.declare_dram_parameter("posT", [steps, 2, BS], F32, isOutput=True)

    with TileContext(nc) as tc:
        with (
            tc.tile_pool(name="wpool", bufs=1) as wp,
            tc.tile_pool(name="state", bufs=1) as st,
            tc.tile_pool(name="work", bufs=2) as wk,
            tc.tile_pool(name="psum", bufs=2, space="PSUM") as ps,
        ):
            # ---- load weights / constants ----
            # ev/w_init first so h0 can start ~1us in; w_hh streams during h0
            winit = []
            ev = []
            for k in range(KT):
                wt_ = wp.tile([128, H], F16, tag=f"winit{k}", name=f"winit{k}")
                nc.sync.dma_start(out=wt_[:], in_=w_initT[k * 128 : (k + 1) * 128, :])
                winit.append(wt_)
                et_ = wp.tile([128, BS], F16, tag=f"ev{k}", name=f"ev{k}")
                nc.sync.dma_start(out=et_[:], in_=evT[k * 128 : (k + 1) * 128, :])
                ev.append(et_)
            # w_ih / w_out zero-padded to full 128-row / 128-col stationary
            # tiles so every matmul keeps the same (128,128) PE geometry —
            # mixed-geometry LDWEIGHTS can't overlap a running matmul and
            # costs ~100ns per transition.
            biases = {}
            for nm, src in [
                ("b_init", b_init_t),
                ("b_r", b_r_t),
                ("b_z", b_z_t),
                ("b_hn", b_hn_t),
                ("b_in", b_in_t),
            ]:
                t_ = wp.tile([128, MT], F32, tag=nm, name=nm)
                nc.sync.dma_start(out=t_[:], in_=src[:])
                biases[nm] = t_
            bout = wp.tile([2, 1], F32, tag="bout")
            nc.sync.dma_start(out=bout[:], in_=b_out_c[:])

            # teacher-input staging: full 128-partition tiles, rows 2..127
            # stay zero, rows 0..1 refreshed per step from DRAM
            tstage = []
            for p in range(2):
                t_ = wp.tile([128, BS], F16, tag=f"tstage{p}", name=f"tstage{p}")
                nc.vector.memset(t_[:], 0.0)
                tstage.append(t_)

            wih = wp.tile([128, 3 * H], F16, tag="wih")
            nc.sync.dma_start(out=wih[:], in_=w_ihT[:])

            # w_hh in j-major blocks (384 cols = [r_j | z_j | n_j]) streamed
            # j-outer so step-1 m-tiles can begin before the full load
            whh = [
                wp.tile([128, 3 * H], F16, tag=f"whh{k}", name=f"whh{k}")
                for k in range(KT)
            ]
            for j in range(MT):
                for k in range(KT):
                    b = (j * KT + k) * 128
                    nc.sync.dma_start(
                        out=whh[k][:, j * 384 : (j + 1) * 384],
                        in_=w_hhT[b : b + 128, :],
                    )
            wout = wp.tile([128, 128 * KT], F16, tag="wout")
            nc.sync.dma_start(out=wout[:], in_=woutKM[:])

            # hidden state, ping-pong
            h16 = [
                [
                    st.tile([128, BS], F16, tag=f"h{p}_{j}", name=f"h{p}_{j}")
                    for j in range(MT)
                ]
                for p in range(2)
            ]
            posacc = st.tile([2, BS], F32, tag="posacc")
            nc.vector.memset(posacc[:], 0.0)

            # ---- h0 = tanh(w_init.T-matmul + b_init) ----
            # k-outer across all 8 PSUM banks so the matmuls stream in DMA
            # arrival order of the (w_init, ev) k-tile pairs
            h0_tags = [
                ("r", 2), ("r", 2), ("z", 1), ("hn", 2),
                ("hn", 2), ("xn", 2), ("xn", 2), ("delta", 1),
            ]
            h0acc = [
                ps.tile([128, BS], F32, tag=tg, bufs=bf, name=f"h0acc{j}")
                for j, (tg, bf) in enumerate(h0_tags)
            ]
            for k in range(KT):
                for j in range(MT):
                    nc.tensor.matmul(
                        h0acc[j][:],
                        winit[k][:, j * 128 : (j + 1) * 128],
                        ev[k][:],
                        start=(k == 0),
                        stop=(k == KT - 1),
                        skip_group_check=True,
                    )
            for j in range(MT):
                nc.scalar.activation(
                    h16[0][j][:],
                    h0acc[j][:],
                    AF.Tanh,
                    bias=biases["b_init"][:, j : j + 1],
                )

            # ---- recurrence ----
            for t in range(steps):
                p = t % 2
                cur, nxt = h16[p], h16[p ^ 1]
                tch = tstage[p]
                nc.sync.dma_start(
                    out=tch[0:2, :], in_=teachT[:, t * BS : (t + 1) * BS]
                )
                for j in range(MT):
                    ps_r = ps.tile([128, BS], F32, tag="r", bufs=2)
                    for k in range(KT):
                        nc.tensor.matmul(
                            ps_r[:],
                            whh[k][:, j * 384 : j * 384 + 128],
                            cur[k][:],
                            start=(k == 0),
                            stop=False,
                        )
                    nc.tensor.matmul(
                        ps_r[:], wih[:, j * 128 : (j + 1) * 128], tch[:],
                        start=False, stop=True,
                    )
                    ps_z = ps.tile([128, BS], F32, tag="z", bufs=1)
                    for k in range(KT):
                        nc.tensor.matmul(
                            ps_z[:],
                            whh[k][:, j * 384 + 128 : j * 384 + 256],
                            cur[k][:],
                            start=(k == 0),
                            stop=False,
                        )
                    nc.tensor.matmul(
                        ps_z[:], wih[:, H + j * 128 : H + (j + 1) * 128], tch[:],
                        start=False, stop=True,
                    )
                    ps_hn = ps.tile([128, BS], F32, tag="hn", bufs=2)
                    for k in range(KT):
                        nc.tensor.matmul(
                            ps_hn[:],
                            whh[k][:, j * 384 + 256 : j * 384 + 384],
                            cur[k][:],
                            start=(k == 0),
                            stop=(k == KT - 1),
                        )
                    ps_xn = ps.tile([128, BS], F32, tag="xn", bufs=2)
                    nc.tensor.matmul(
                        ps_xn[:], wih[:, 2 * H + j * 128 : 2 * H + (j + 1) * 128],
                        tch[:], start=True, stop=True,
                    )

                    # gates
                    r16 = wk.tile([128, BS], F16, tag="r16")
                    nc.scalar.activation(
                        r16[:], ps_r[:], AF.Sigmoid, bias=biases["b_r"][:, j : j + 1]
                    )
                    z16 = wk.tile([128, BS], F16, tag="z16")
                    nc.scalar.activation(
                        z16[:], ps_z[:], AF.Sigmoid, bias=biases["b_z"][:, j : j + 1]
                    )
                    rhn = wk.tile([128, BS], F32, tag="rhn")
                    nc.vector.scalar_tensor_tensor(
                        rhn[:], ps_hn[:], biases["b_hn"][:, j : j + 1], r16[:],
                        op0=ALU.add, op1=ALU.mult,
                    )
                    npre = wk.tile([128, BS], F32, tag="npre")
                    nc.vector.tensor_add(npre[:], rhn[:], ps_xn[:])
                    n16 = wk.tile([128, BS], F16, tag="n16")
                    nc.scalar.activation(
                        n16[:], npre[:], AF.Tanh, bias=biases["b_in"][:, j : j + 1]
                    )
                    d_ = wk.tile([128, BS], F32, tag="d_")
                    nc.vector.tensor_sub(d_[:], cur[j][:], n16[:])
                    e_ = wk.tile([128, BS], F32, tag="e_")
                    nc.vector.tensor_mul(e_[:], z16[:], d_[:])
                    nc.vector.tensor_add(nxt[j][:], n16[:], e_[:])

                # delta = h_new @ w_out.T (contract over the 8 hidden k-tiles;
                # wout zero-padded to M=128, only partitions 0..1 meaningful)
                ps_d = ps.tile([128, BS], F32, tag="delta", bufs=1)
                for j in range(MT):
                    nc.tensor.matmul(
                        ps_d[:], wout[:, j * 128 : (j + 1) * 128], nxt[j][:],
                        start=(j == 0), stop=(j == MT - 1),
                    )

                # delta + bias, position accumulate, write out
                dstage = wk.tile([2, BS], F32, tag="dstage")
                nc.scalar.activation(
                    dstage[:], ps_d[0:2, :], AF.Identity, bias=bout[:, 0:1]
                )
                nc.vector.tensor_add(posacc[:], posacc[:], dstage[:])
                nc.sync.dma_start(out=deltasT[t], in_=dstage[:])
                nc.sync.dma_start(out=posT[t], in_=posacc[:])

    _split_waits(nc)
    return nc


def _get_program(steps=T):
    global _PROGRAM
    if _PROGRAM is None or _PROGRAM[1] != steps:
        _PROGRAM = (_build(steps), steps)
    return _PROGRAM[0]


def _prep_core_inputs(ev_c, teach_c, shared):
    m = dict(shared)
    m["evT"] = np.ascontiguousarray(ev_c.T).astype(np.float16)
    steps = teach_c.shape[1]
    tt = np.zeros((2, steps, BS), np.float16)
    tt[:, 1:, :] = teach_c[:, : steps - 1, :].transpose(2, 1, 0)
    m["teachT"] = tt.reshape(2, steps * BS)
    return m


def _shared_inputs(w_init, b_init, w_ih, w_hh, b_ih, b_hh, w_out, b_out):
    f16 = np.float16
    f32 = np.float32
    b_rz = (b_ih + b_hh).astype(f32)
    w_ih_pad = np.zeros((128, 3 * H), f16)
    w_ih_pad[0:2, :] = w_ih.T.astype(f16)
    wout_pad = np.zeros((128, 128 * KT), f16)
    for k in range(KT):
        wout_pad[:, k * 128 : k * 128 + 2] = w_out.T[k * 128 : (k + 1) * 128].astype(
            f16
        )
    # w_hh.T packed as contiguous (j, k) chunks of [128, 384] = [r_j|z_j|n_j]
    whhT = w_hh.T.astype(f16)
    whh_j = np.concatenate(
        [
            np.concatenate(
                [
                    whhT[k * 128 : (k + 1) * 128, j * 128 : (j + 1) * 128],
                    whhT[k * 128 : (k + 1) * 128, H + j * 128 : H + (j + 1) * 128],
                    whhT[k * 128 : (k + 1) * 128, 2 * H + j * 128 : 2 * H + (j + 1) * 128],
                ],
                axis=1,
            )
            for j in range(MT)
            for k in range(KT)
        ],
        axis=0,
    )
    shared = {
        "w_initT": np.ascontiguousarray(w_init.T).astype(f16),
        "w_hhT": np.ascontiguousarray(whh_j),
        "w_ihT": w_ih_pad,
        "woutKM": wout_pad,
        "b_init_t": np.ascontiguousarray(b_init.reshape(MT, 128).T).astype(f32),
        "b_r_t": np.ascontiguousarray(b_rz[:H].reshape(MT, 128).T),
        "b_z_t": np.ascontiguousarray(b_rz[H : 2 * H].reshape(MT, 128).T),
        "b_hn_t": np.ascontiguousarray(b_hh[2 * H :].reshape(MT, 128).T).astype(f32),
        "b_in_t": np.ascontiguousarray(b_ih[2 * H :].reshape(MT, 128).T).astype(f32),
        "b_out_c": b_out.reshape(2, 1).astype(f32),
    }
    return shared


def kernel(
    ev_context,
    teacher_deltas,
    w_init,
    b_init,
    w_ih,
    w_hh,
    b_ih,
    b_hh,
    w_out,
    b_out,
    steps=T,
    trace=False,
):
    from concourse.bass_utils import run_bass_kernel_spmd

    ev_context = np.asarray(ev_context, np.float32)
    teacher_deltas = np.asarray(teacher_deltas, np.float32)

    nc = _get_program(steps)
    shared = _shared_inputs(
        np.asarray(w_init, np.float32),
        np.asarray(b_init, np.float32),
        np.asarray(w_ih, np.float32),
        np.asarray(w_hh, np.float32),
        np.asarray(b_ih, np.float32),
        np.asarray(b_hh, np.float32),
        np.asarray(w_out, np.float32),
        np.asarray(b_out, np.float32),
    )
    in_maps = []
    for c in range(NCORES):
        sl = slice(c * BS, (c + 1) * BS)
        in_maps.append(
            _prep_core_inputs(ev_context[sl], teacher_deltas[sl, :steps], shared)
        )

    res = run_bass_kernel_spmd(nc, in_maps, list(range(NCORES)), trace=trace)

    pred_deltas = np.empty((B, steps, 2), np.float32)
    pred_pos = np.empty((B, steps, 2), np.float32)
    for c in range(NCORES):
        sl = slice(c * BS, (c + 1) * BS)
        pred_deltas[sl] = res.results[c]["deltasT"].transpose(2, 0, 1)
        pred_pos[sl] = res.results[c]["posT"].transpose(2, 0, 1)
    if trace:
        return (pred_deltas, pred_pos), res
    return pred_deltas, pred_pos


# revision 37
# speedup vs baseline: 1.0070x; 1.0070x over previous
"""GRU trajectory head on 8 Trainium2 NeuronCores.

Problem: B=4096, D=1024, H=1024, T=50, FREE_RATIO=0.0 (teacher forcing
always wins, so the GRU input at step t is teacher_deltas[:, t-1] and
step 0 input is zeros — the recurrence only flows through h).

Sharding: data-parallel over batch, 512 rows per core; weights replicated.

Per-core layout (feature dim on partitions, batch on free dim):
  h16[j]      [128, 512] fp16, j=0..7  (hidden state, ping-pong buffered)
  gh psum     [128, 512] accumulates  w_hhT k-tiles (8 fp16 matmuls)
                                      + w_ihT K=2 matmul (teacher input)
  gates       ACT sigmoid/tanh with per-partition bias, DVE for products
  delta       [2, 512] psum via w_out k-tile matmuls on the new h
"""

import sys

sys.path.insert(0, "/opt/trn_rl_repo")

import numpy as np

B, D, H, T = 4096, 1024, 1024, 50
NCORES = 8
BS = B // NCORES  # 512 batch rows per core
KT = H // 128  # 8 k-tiles over the hidden dim
MT = H // 128  # 8 m-tiles per gate

_PROGRAM = None


def _split_waits(nc, limit=1):
    # This walrus build allows few sync-wait commands per instruction
    # (1 on ACT); hoist excess waits onto same-engine NoOps just before.
    import concourse.mybir as mybir

    for fn in nc.m.functions:
        for bb in list(fn.blocks):
            new_list = []
            changed = False
            for inst in bb.instructions:
                si = getattr(inst, "sync_info", None)
                waits = list(si.on_wait) if si is not None and si.on_wait else []
                if len(waits) > limit:
                    changed = True
                    excess, keep = waits[:-limit], waits[-limit:]
                    for j in range(0, len(excess), limit):
                        nop = mybir.InstNoOp(
                            name=nc.get_next_instruction_name(),
                            engine=inst.engine,
                            bass_nofuse=True,
                            sync_info=mybir.SyncInfo(
                                on_wait=excess[j : j + limit], on_update=[]
                            ),
                        )
                        nc.register_instruction(nop)
                        new_list.append(nop)
                    inst.sync_info = mybir.SyncInfo(on_wait=keep, on_update=si.on_update)
                new_list.append(inst)
            if changed:
                bb.instructions = new_list


def _build(steps=T):
    import concourse.bass as bass
    import concourse.mybir as mybir
    from concourse.tile import TileContext

    F32 = mybir.dt.float32
    F16 = mybir.dt.float16
    AF = mybir.ActivationFunctionType
    ALU = mybir.AluOpType

    nc = bass.Bass()

    # ewPack row-block k = [w_init.T k-tile | ev.T k-tile]; whhPack row-block
    # j = [128, KT*384] with free dim (k, [r_j|z_j|n_j]); biasPack cols =
    # [b_init | b_r | b_z | b_hn | b_in | b_out] — few big contiguous DMAs
    # (DMA dispatch costs ~600ns per instruction on the queue).
    ewPack = nc.declare_dram_parameter("ewPack", [KT * 128, H + BS], F16, isOutput=False)
    teachT = nc.declare_dram_parameter("teachT", [2, steps * BS], F16, isOutput=False)
    whhPack = nc.declare_dram_parameter(
        "whhPack", [MT * 128, KT * 384], F16, isOutput=False
    )
    w_ihT = nc.declare_dram_parameter("w_ihT", [128, 3 * H], F16, isOutput=False)
    woutKM = nc.declare_dram_parameter("woutKM", [128, 128 * KT], F16, isOutput=False)
    biasPack = nc.declare_dram_parameter("biasPack", [128, 5 * MT + 1], F32, isOutput=False)
    deltasT = nc.declare_dram_parameter("deltasT", [steps, 2, BS], F32, isOutput=True)
    posT = nc.declare_dram_parameter("posT", [steps, 2, BS], F32, isOutput=True)

    with TileContext(nc) as tc:
        with (
            tc.tile_pool(name="wpool", bufs=1) as wp,
            tc.tile_pool(name="state", bufs=1) as st,
            tc.tile_pool(name="work", bufs=2) as wk,
            tc.tile_pool(name="psum", bufs=2, space="PSUM") as ps,
        ):
            # ---- load weights / constants ----
            # (w_init_k | ev_k) pairs first, in h0's k-consumption order
            ew = []
            for k in range(KT):
                t_ = wp.tile([128, H + BS], F16, tag=f"ew{k}", name=f"ew{k}")
                nc.sync.dma_start(out=t_[:], in_=ewPack[k * 128 : (k + 1) * 128, :])
                ew.append(t_)
            winit = [ew[k][:, 0:H] for k in range(KT)]
            ev = [ew[k][:, H : H + BS] for k in range(KT)]

            # w_hh per-j blocks, j-outer so step 1 can begin before full load;
            # whhJ[j] free layout is (k, [r_j | z_j | n_j])
            whhJ = []
            for j in range(MT):
                t_ = wp.tile([128, KT * 384], F16, tag=f"whhJ{j}", name=f"whhJ{j}")
                nc.sync.dma_start(out=t_[:], in_=whhPack[j * 128 : (j + 1) * 128, :])
                whhJ.append(t_)

            # w_ih / w_out zero-padded to full 128-row / 128-col stationary
            # tiles so every matmul keeps the same (128,128) PE geometry —
            # mixed-geometry LDWEIGHTS can't overlap a running matmul and
            # costs ~100ns per transition.
            wih = wp.tile([128, 3 * H], F16, tag="wih")
            nc.sync.dma_start(out=wih[:], in_=w_ihT[:])
            wout = wp.tile([128, 128 * KT], F16, tag="wout")
            nc.sync.dma_start(out=wout[:], in_=woutKM[:])

            bias = wp.tile([128, 5 * MT + 1], F32, tag="bias")
            nc.sync.dma_start(out=bias[:], in_=biasPack[:])
            biases = {
                nm: bias[:, i * MT : (i + 1) * MT]
                for i, nm in enumerate(["b_init", "b_r", "b_z", "b_hn", "b_in"])
            }
            bout = bias[0:2, 5 * MT : 5 * MT + 1]

            # teacher-input staging: full 128-partition tiles, rows 2..127
            # stay zero, rows 0..1 refreshed per step from DRAM
            tstage = []
            for p in range(2):
                t_ = wp.tile([128, BS], F16, tag=f"tstage{p}", name=f"tstage{p}")
                nc.vector.memset(t_[:], 0.0)
                tstage.append(t_)

            # hidden state, ping-pong
            h16 = [
                [
                    st.tile([128, BS], F16, tag=f"h{p}_{j}", name=f"h{p}_{j}")
                    for j in range(MT)
                ]
                for p in range(2)
            ]
            posacc = st.tile([2, BS], F32, tag="posacc")
            nc.vector.memset(posacc[:], 0.0)

            # ---- h0 = tanh(w_init.T-matmul + b_init) ----
            # k-outer across all 8 PSUM banks so the matmuls stream in DMA
            # arrival order of the (w_init, ev) k-tile pairs
            h0_tags = [
                ("r", 2), ("r", 2), ("z", 1), ("hn", 2),
                ("hn", 2), ("xn", 2), ("xn", 2), ("delta", 1),
            ]
            h0acc = [
                ps.tile([128, BS], F32, tag=tg, bufs=bf, name=f"h0acc{j}")
                for j, (tg, bf) in enumerate(h0_tags)
            ]
            for k in range(KT):
                for j in range(MT):
                    nc.tensor.matmul(
                        h0acc[j][:],
                        winit[k][:, j * 128 : (j + 1) * 128],
                        ev[k][:],
                        start=(k == 0),
                        stop=(k == KT - 1),
                        skip_group_check=True,
                    )
            for j in range(MT):
                nc.scalar.activation(
                    h16[0][j][:],
                    h0acc[j][:],
                    AF.Tanh,
                    bias=biases["b_init"][:, j : j + 1],
                )

            # ---- recurrence ----
            for t in range(steps):
                p = t % 2
                cur, nxt = h16[p], h16[p ^ 1]
                tch = tstage[p]
                nc.sync.dma_start(
                    out=tch[0:2, :], in_=teachT[:, t * BS : (t + 1) * BS]
                )
                for j in range(MT):
                    ps_r = ps.tile([128, BS], F32, tag="r", bufs=2)
                    for k in range(KT):
                        nc.tensor.matmul(
                            ps_r[:],
                            whhJ[j][:, k * 384 : k * 384 + 128],
                            cur[k][:],
                            start=(k == 0),
                            stop=False,
                        )
                    nc.tensor.matmul(
                        ps_r[:], wih[:, j * 128 : (j + 1) * 128], tch[:],
                        start=False, stop=True,
                    )
                    ps_z = ps.tile([128, BS], F32, tag="z", bufs=1)
                    for k in range(KT):
                        nc.tensor.matmul(
                            ps_z[:],
                            whhJ[j][:, k * 384 + 128 : k * 384 + 256],
                            cur[k][:],
                            start=(k == 0),
                            stop=False,
                        )
                    nc.tensor.matmul(
                        ps_z[:], wih[:, H + j * 128 : H + (j + 1) * 128], tch[:],
                        start=False, stop=True,
                    )
                    ps_hn = ps.tile([128, BS], F32, tag="hn", bufs=2)
                    for k in range(KT):
                        nc.tensor.matmul(
                            ps_hn[:],
                            whhJ[j][:, k * 384 + 256 : k * 384 + 384],
                            cur[k][:],
                            start=(k == 0),
                            stop=(k == KT - 1),
                        )
                    ps_xn = ps.tile([128, BS], F32, tag="xn", bufs=2)
                    nc.tensor.matmul(
                        ps_xn[:], wih[:, 2 * H + j * 128 : 2 * H + (j + 1) * 128],
                        tch[:], start=True, stop=True,
                    )

                    # gates
                    r16 = wk.tile([128, BS], F16, tag="r16")
                    nc.scalar.activation(
                        r16[:], ps_r[:], AF.Sigmoid, bias=biases["b_r"][:, j : j + 1]
                    )
                    z16 = wk.tile([128, BS], F16, tag="z16")
                    nc.scalar.activation(
                        z16[:], ps_z[:], AF.Sigmoid, bias=biases["b_z"][:, j : j + 1]
                    )
                    rhn = wk.tile([128, BS], F32, tag="rhn")
                    nc.vector.scalar_tensor_tensor(
                        rhn[:], ps_hn[:], biases["b_hn"][:, j : j + 1], r16[:],
                        op0=ALU.add, op1=ALU.mult,
                    )
                    npre = wk.tile([128, BS], F32, tag="npre")
                    nc.vector.tensor_add(npre[:], rhn[:], ps_xn[:])
                    n16 = wk.tile([128, BS], F16, tag="n16")
                    nc.scalar.activation(
                        n16[:], npre[:], AF.Tanh, bias=biases["b_in"][:, j : j + 1]
                    )
                    d_ = wk.tile([128, BS], F32, tag="d_")
                    nc.vector.tensor_sub(d_[:], cur[j][:], n16[:])
                    e_ = wk.tile([128, BS], F32, tag="e_")
                    nc.vector.tensor_mul(e_[:], z16[:], d_[:])
                    nc.vector.tensor_add(nxt[j][:], n16[:], e_[:])

                # delta = h_new @ w_out.T (contract over the 8 hidden k-tiles;
                # wout zero-padded to M=128, only partitions 0..1 meaningful)
                ps_d = ps.tile([128, BS], F32, tag="delta", bufs=1)
                for j in range(MT):
                    nc.tensor.matmul(
                        ps_d[:], wout[:, j * 128 : (j + 1) * 128], nxt[j][:],
                        start=(j == 0), stop=(j == MT - 1),
                    )

                # delta + bias, position accumulate, write out
                dstage = wk.tile([2, BS], F32, tag="dstage")
                nc.scalar.activation(dstage[:], ps_d[0:2, :], AF.Identity, bias=bout)
                nc.vector.tensor_add(posacc[:], posacc[:], dstage[:])
                nc.sync.dma_start(out=deltasT[t], in_=dstage[:])
                nc.sync.dma_start(out=posT[t], in_=posacc[:])

    _split_waits(nc)
    return nc


def _get_program(steps=T):
    global _PROGRAM
    if _PROGRAM is None or _PROGRAM[1] != steps:
        _PROGRAM = (_build(steps), steps)
    return _PROGRAM[0]


def _prep_core_inputs(ev_c, teach_c, shared, winitT16):
    m = dict(shared)
    ew = np.empty((KT * 128, H + BS), np.float16)
    ew[:, 0:H] = winitT16
    ew[:, H:] = ev_c.T.astype(np.float16).reshape(KT * 128, BS)
    m["ewPack"] = ew
    steps = teach_c.shape[1]
    tt = np.zeros((2, steps, BS), np.float16)
    tt[:, 1:, :] = teach_c[:, : steps - 1, :].transpose(2, 1, 0)
    m["teachT"] = tt.reshape(2, steps * BS)
    return m


def _shared_inputs(w_init, b_init, w_ih, w_hh, b_ih, b_hh, w_out, b_out):
    f16 = np.float16
    f32 = np.float32
    b_rz = (b_ih + b_hh).astype(f32)
    w_ih_pad = np.zeros((128, 3 * H), f16)
    w_ih_pad[0:2, :] = w_ih.T.astype(f16)
    wout_pad = np.zeros((128, 128 * KT), f16)
    for k in range(KT):
        wout_pad[:, k * 128 : k * 128 + 2] = w_out.T[k * 128 : (k + 1) * 128].astype(
            f16
        )
    # whhPack row-block j = [128 partitions, (k, [r_j|z_j|n_j])]
    whhT = w_hh.T.astype(f16)  # [H, 3H]
    whh_pack = np.empty((MT * 128, KT * 384), f16)
    for j in range(MT):
        for k in range(KT):
            rows = slice(k * 128, (k + 1) * 128)
            whh_pack[j * 128 : (j + 1) * 128, k * 384 + 0 : k * 384 + 128] = whhT[
                rows, j * 128 : (j + 1) * 128
            ]
            whh_pack[j * 128 : (j + 1) * 128, k * 384 + 128 : k * 384 + 256] = whhT[
                rows, H + j * 128 : H + (j + 1) * 128
            ]
            whh_pack[j * 128 : (j + 1) * 128, k * 384 + 256 : k * 384 + 384] = whhT[
                rows, 2 * H + j * 128 : 2 * H + (j + 1) * 128
            ]

    bias_pack = np.zeros((128, 5 * MT + 1), f32)
    bias_pack[:, 0:MT] = b_init.reshape(MT, 128).T
    bias_pack[:, MT : 2 * MT] = b_rz[:H].reshape(MT, 128).T
    bias_pack[:, 2 * MT : 3 * MT] = b_rz[H : 2 * H].reshape(MT, 128).T
    bias_pack[:, 3 * MT : 4 * MT] = b_hh[2 * H :].reshape(MT, 128).T
    bias_pack[:, 4 * MT : 5 * MT] = b_ih[2 * H :].reshape(MT, 128).T
    bias_pack[0:2, 5 * MT] = b_out

    shared = {
        "whhPack": whh_pack,
        "w_ihT": w_ih_pad,
        "woutKM": wout_pad,
        "biasPack": bias_pack,
    }
    return shared


def kernel(
    ev_context,
    teacher_deltas,
    w_init,
    b_init,
    w_ih,
    w_hh,
    b_ih,
    b_hh,
    w_out,
    b_out,
    steps=T,
    trace=False,
):
    from concourse.bass_utils import run_bass_kernel_spmd

    ev_context = np.asarray(ev_context, np.float32)
    teacher_deltas = np.asarray(teacher_deltas, np.float32)

    nc = _get_program(steps)
    shared = _shared_inputs(
        np.asarray(w_init, np.float32),
        np.asarray(b_init, np.float32),
        np.asarray(w_ih, np.float32),
        np.asarray(w_hh, np.float32),
        np.asarray(b_ih, np.float32),
        np.asarray(b_hh, np.float32),
        np.asarray(w_out, np.float32),
        np.asarray(b_out, np.float32),
    )
    winitT16 = np.ascontiguousarray(np.asarray(w_init, np.float32).T).astype(np.float16)
    in_maps = []
    for c in range(NCORES):
        sl = slice(c * BS, (c + 1) * BS)
        in_maps.append(
            _prep_core_inputs(
                ev_context[sl], teacher_deltas[sl, :steps], shared, winitT16
            )
        )

    res = run_bass_kernel_spmd(nc, in_maps, list(range(NCORES)), trace=trace)

    pred_deltas = np.empty((B, steps, 2), np.float32)
    pred_pos = np.empty((B, steps, 2), np.float32)
    for c in range(NCORES):
        sl = slice(c * BS, (c + 1) * BS)
        pred_deltas[sl] = res.results[c]["deltasT"].transpose(2, 0, 1)
        pred_pos[sl] = res.results[c]["posT"].transpose(2, 0, 1)
    if trace:
        return (pred_deltas, pred_pos), res
    return pred_deltas, pred_pos


# revision 39
# speedup vs baseline: 1.0132x; 1.0061x over previous
"""GRU trajectory head on 8 Trainium2 NeuronCores.

Problem: B=4096, D=1024, H=1024, T=50, FREE_RATIO=0.0 (teacher forcing
always wins, so the GRU input at step t is teacher_deltas[:, t-1] and
step 0 input is zeros — the recurrence only flows through h).

Sharding: data-parallel over batch, 512 rows per core; weights replicated.

Per-core layout (feature dim on partitions, batch on free dim):
  h16[j]      [128, 512] fp16, j=0..7  (hidden state, ping-pong buffered)
  gh psum     [128, 512] accumulates  w_hhT k-tiles (8 fp16 matmuls)
                                      + w_ihT K=2 matmul (teacher input)
  gates       ACT sigmoid/tanh with per-partition bias, DVE for products
  delta       [2, 512] psum via w_out k-tile matmuls on the new h
"""

import sys

sys.path.insert(0, "/opt/trn_rl_repo")

import numpy as np

B, D, H, T = 4096, 1024, 1024, 50
NCORES = 8
BS = B // NCORES  # 512 batch rows per core
KT = H // 128  # 8 k-tiles over the hidden dim
MT = H // 128  # 8 m-tiles per gate

_PROGRAM = None


def _split_waits(nc, limit=1):
    # This walrus build allows few sync-wait commands per instruction
    # (1 on ACT); hoist excess waits onto same-engine NoOps just before.
    import concourse.mybir as mybir

    for fn in nc.m.functions:
        for bb in list(fn.blocks):
            new_list = []
            changed = False
            for inst in bb.instructions:
                si = getattr(inst, "sync_info", None)
                waits = list(si.on_wait) if si is not None and si.on_wait else []
                if len(waits) > limit:
                    changed = True
                    excess, keep = waits[:-limit], waits[-limit:]
                    for j in range(0, len(excess), limit):
                        nop = mybir.InstNoOp(
                            name=nc.get_next_instruction_name(),
                            engine=inst.engine,
                            bass_nofuse=True,
                            sync_info=mybir.SyncInfo(
                                on_wait=excess[j : j + limit], on_update=[]
                            ),
                        )
                        nc.register_instruction(nop)
                        new_list.append(nop)
                    inst.sync_info = mybir.SyncInfo(on_wait=keep, on_update=si.on_update)
                new_list.append(inst)
            if changed:
                bb.instructions = new_list


def _build(steps=T):
    import concourse.bass as bass
    import concourse.mybir as mybir
    from concourse.tile import TileContext

    F32 = mybir.dt.float32
    F16 = mybir.dt.float16
    AF = mybir.ActivationFunctionType
    ALU = mybir.AluOpType

    nc = bass.Bass()

    # ewPack row-block k = [w_init.T k-tile | ev.T k-tile]; whhPack row-block
    # j = [128, KT*384] with free dim (k, [r_j|z_j|n_j]); biasPack cols =
    # [b_init | b_r | b_z | b_hn | b_in | b_out] — few big contiguous DMAs
    # (DMA dispatch costs ~600ns per instruction on the queue).
    ewPack = nc.declare_dram_parameter("ewPack", [KT * 128, H + BS], F16, isOutput=False)
    teachT = nc.declare_dram_parameter("teachT", [2, steps * BS], F16, isOutput=False)
    whhPack = nc.declare_dram_parameter(
        "whhPack", [MT * 128, KT * 384], F16, isOutput=False
    )
    w_ihT = nc.declare_dram_parameter("w_ihT", [128, 3 * H], F16, isOutput=False)
    woutKM = nc.declare_dram_parameter("woutKM", [128, 128 * KT], F16, isOutput=False)
    biasPack = nc.declare_dram_parameter("biasPack", [128, 5 * MT + 1], F32, isOutput=False)
    deltasT = nc.declare_dram_parameter("deltasT", [steps, 2, BS], F32, isOutput=True)
    posT = nc.declare_dram_parameter("posT", [steps, 2, BS], F32, isOutput=True)

    with TileContext(nc) as tc:
        with (
            tc.tile_pool(name="wpool", bufs=1) as wp,
            tc.tile_pool(name="state", bufs=1) as st,
            tc.tile_pool(name="work", bufs=2) as wk,
            tc.tile_pool(name="psum", bufs=2, space="PSUM") as ps,
        ):
            # ---- load weights / constants ----
            # biases first (tiny; h0's tanh needs them right after the first
            # matmul group), then (w_init_k | ev_k) pairs in h0's k-order
            bias = wp.tile([128, 5 * MT + 1], F32, tag="bias")
            nc.sync.dma_start(out=bias[:], in_=biasPack[:])
            biases = {
                nm: bias[:, i * MT : (i + 1) * MT]
                for i, nm in enumerate(["b_init", "b_r", "b_z", "b_hn", "b_in"])
            }
            bout = bias[0:2, 5 * MT : 5 * MT + 1]

            ew = []
            for k in range(KT):
                t_ = wp.tile([128, H + BS], F16, tag=f"ew{k}", name=f"ew{k}")
                nc.sync.dma_start(out=t_[:], in_=ewPack[k * 128 : (k + 1) * 128, :])
                ew.append(t_)
            winit = [ew[k][:, 0:H] for k in range(KT)]
            ev = [ew[k][:, H : H + BS] for k in range(KT)]

            # w_hh per-j blocks, j-outer so step 1 can begin before full load;
            # whhJ[j] free layout is (k, [r_j | z_j | n_j])
            whhJ = []
            for j in range(MT):
                t_ = wp.tile([128, KT * 384], F16, tag=f"whhJ{j}", name=f"whhJ{j}")
                nc.sync.dma_start(out=t_[:], in_=whhPack[j * 128 : (j + 1) * 128, :])
                whhJ.append(t_)

            # w_ih / w_out zero-padded to full 128-row / 128-col stationary
            # tiles so every matmul keeps the same (128,128) PE geometry —
            # mixed-geometry LDWEIGHTS can't overlap a running matmul and
            # costs ~100ns per transition.
            wih = wp.tile([128, 3 * H], F16, tag="wih")
            nc.sync.dma_start(out=wih[:], in_=w_ihT[:])
            wout = wp.tile([128, 128 * KT], F16, tag="wout")
            nc.sync.dma_start(out=wout[:], in_=woutKM[:])

            # teacher-input staging: full 128-partition tiles, rows 2..127
            # stay zero, rows 0..1 refreshed per step from DRAM
            tstage = []
            for p in range(2):
                t_ = wp.tile([128, BS], F16, tag=f"tstage{p}", name=f"tstage{p}")
                nc.vector.memset(t_[:], 0.0)
                tstage.append(t_)

            # hidden state, ping-pong
            h16 = [
                [
                    st.tile([128, BS], F16, tag=f"h{p}_{j}", name=f"h{p}_{j}")
                    for j in range(MT)
                ]
                for p in range(2)
            ]
            posacc = st.tile([2, BS], F32, tag="posacc")
            nc.vector.memset(posacc[:], 0.0)

            # ---- h0 = tanh(w_init.T-matmul + b_init) ----
            # k-outer across all 8 PSUM banks so the matmuls stream in DMA
            # arrival order of the (w_init, ev) k-tile pairs
            h0_tags = [
                ("r", 2), ("r", 2), ("z", 1), ("hn", 2),
                ("hn", 2), ("xn", 2), ("xn", 2), ("delta", 1),
            ]
            h0acc = [
                ps.tile([128, BS], F32, tag=tg, bufs=bf, name=f"h0acc{j}")
                for j, (tg, bf) in enumerate(h0_tags)
            ]
            for k in range(KT):
                for j in range(MT):
                    nc.tensor.matmul(
                        h0acc[j][:],
                        winit[k][:, j * 128 : (j + 1) * 128],
                        ev[k][:],
                        start=(k == 0),
                        stop=(k == KT - 1),
                        skip_group_check=True,
                    )
            for j in range(MT):
                nc.scalar.activation(
                    h16[0][j][:],
                    h0acc[j][:],
                    AF.Tanh,
                    bias=biases["b_init"][:, j : j + 1],
                )

            # ---- recurrence ----
            for t in range(steps):
                p = t % 2
                cur, nxt = h16[p], h16[p ^ 1]
                tch = tstage[p]
                nc.sync.dma_start(
                    out=tch[0:2, :], in_=teachT[:, t * BS : (t + 1) * BS]
                )
                for j in range(MT):
                    ps_r = ps.tile([128, BS], F32, tag="r", bufs=2)
                    for k in range(KT):
                        nc.tensor.matmul(
                            ps_r[:],
                            whhJ[j][:, k * 384 : k * 384 + 128],
                            cur[k][:],
                            start=(k == 0),
                            stop=False,
                        )
                    nc.tensor.matmul(
                        ps_r[:], wih[:, j * 128 : (j + 1) * 128], tch[:],
                        start=False, stop=True,
                    )
                    ps_z = ps.tile([128, BS], F32, tag="z", bufs=1)
                    for k in range(KT):
                        nc.tensor.matmul(
                            ps_z[:],
                            whhJ[j][:, k * 384 + 128 : k * 384 + 256],
                            cur[k][:],
                            start=(k == 0),
                            stop=False,
                        )
                    nc.tensor.matmul(
                        ps_z[:], wih[:, H + j * 128 : H + (j + 1) * 128], tch[:],
                        start=False, stop=True,
                    )
                    ps_hn = ps.tile([128, BS], F32, tag="hn", bufs=2)
                    for k in range(KT):
                        nc.tensor.matmul(
                            ps_hn[:],
                            whhJ[j][:, k * 384 + 256 : k * 384 + 384],
                            cur[k][:],
                            start=(k == 0),
                            stop=(k == KT - 1),
                        )
                    ps_xn = ps.tile([128, BS], F32, tag="xn", bufs=2)
                    nc.tensor.matmul(
                        ps_xn[:], wih[:, 2 * H + j * 128 : 2 * H + (j + 1) * 128],
                        tch[:], start=True, stop=True,
                    )

                    # gates
                    r16 = wk.tile([128, BS], F16, tag="r16")
                    nc.scalar.activation(
                        r16[:], ps_r[:], AF.Sigmoid, bias=biases["b_r"][:, j : j + 1]
                    )
                    z16 = wk.tile([128, BS], F16, tag="z16")
                    nc.scalar.activation(
                        z16[:], ps_z[:], AF.Sigmoid, bias=biases["b_z"][:, j : j + 1]
                    )
                    rhn = wk.tile([128, BS], F32, tag="rhn")
                    nc.vector.scalar_tensor_tensor(
                        rhn[:], ps_hn[:], biases["b_hn"][:, j : j + 1], r16[:],
                        op0=ALU.add, op1=ALU.mult,
                    )
                    npre = wk.tile([128, BS], F32, tag="npre")
                    nc.vector.tensor_add(npre[:], rhn[:], ps_xn[:])
                    n16 = wk.tile([128, BS], F16, tag="n16")
                    nc.scalar.activation(
                        n16[:], npre[:], AF.Tanh, bias=biases["b_in"][:, j : j + 1]
                    )
                    d_ = wk.tile([128, BS], F32, tag="d_")
                    nc.vector.tensor_sub(d_[:], cur[j][:], n16[:])
                    e_ = wk.tile([128, BS], F32, tag="e_")
                    nc.vector.tensor_mul(e_[:], z16[:], d_[:])
                    nc.vector.tensor_add(nxt[j][:], n16[:], e_[:])

                # delta = h_new @ w_out.T (contract over the 8 hidden k-tiles;
                # wout zero-padded to M=128, only partitions 0..1 meaningful)
                ps_d = ps.tile([128, BS], F32, tag="delta", bufs=1)
                for j in range(MT):
                    nc.tensor.matmul(
                        ps_d[:], wout[:, j * 128 : (j + 1) * 128], nxt[j][:],
                        start=(j == 0), stop=(j == MT - 1),
                    )

                # delta + bias, position accumulate, write out
                dstage = wk.tile([2, BS], F32, tag="dstage")
                nc.scalar.activation(dstage[:], ps_d[0:2, :], AF.Identity, bias=bout)
                nc.vector.tensor_add(posacc[:], posacc[:], dstage[:])
                nc.sync.dma_start(out=deltasT[t], in_=dstage[:])
                nc.sync.dma_start(out=posT[t], in_=posacc[:])

    _split_waits(nc)
    return nc


def _get_program(steps=T):
    global _PROGRAM
    if _PROGRAM is None or _PROGRAM[1] != steps:
        _PROGRAM = (_build(steps), steps)
    return _PROGRAM[0]


def _prep_core_inputs(ev_c, teach_c, shared, winitT16):
    m = dict(shared)
    ew = np.empty((KT * 128, H + BS), np.float16)
    ew[:, 0:H] = winitT16
    ew[:, H:] = ev_c.T.astype(np.float16).reshape(KT * 128, BS)
    m["ewPack"] = ew
    steps = teach_c.shape[1]
    tt = np.zeros((2, steps, BS), np.float16)
    tt[:, 1:, :] = teach_c[:, : steps - 1, :].transpose(2, 1, 0)
    m["teachT"] = tt.reshape(2, steps * BS)
    return m


def _shared_inputs(w_init, b_init, w_ih, w_hh, b_ih, b_hh, w_out, b_out):
    f16 = np.float16
    f32 = np.float32
    b_rz = (b_ih + b_hh).astype(f32)
    w_ih_pad = np.zeros((128, 3 * H), f16)
    w_ih_pad[0:2, :] = w_ih.T.astype(f16)
    wout_pad = np.zeros((128, 128 * KT), f16)
    for k in range(KT):
        wout_pad[:, k * 128 : k * 128 + 2] = w_out.T[k * 128 : (k + 1) * 128].astype(
            f16
        )
    # whhPack row-block j = [128 partitions, (k, [r_j|z_j|n_j])]
    whhT = w_hh.T.astype(f16)  # [H, 3H]
    whh_pack = np.empty((MT * 128, KT * 384), f16)
    for j in range(MT):
        for k in range(KT):
            rows = slice(k * 128, (k + 1) * 128)
            whh_pack[j * 128 : (j + 1) * 128, k * 384 + 0 : k * 384 + 128] = whhT[
                rows, j * 128 : (j + 1) * 128
            ]
            whh_pack[j * 128 : (j + 1) * 128, k * 384 + 128 : k * 384 + 256] = whhT[
                rows, H + j * 128 : H + (j + 1) * 128
            ]
            whh_pack[j * 128 : (j + 1) * 128, k * 384 + 256 : k * 384 + 384] = whhT[
                rows, 2 * H + j * 128 : 2 * H + (j + 1) * 128
            ]

    bias_pack = np.zeros((128, 5 * MT + 1), f32)
    bias_pack[:, 0:MT] = b_init.reshape(MT, 128).T
    bias_pack[:, MT : 2 * MT] = b_rz[:H].reshape(MT, 128).T
    bias_pack[:, 2 * MT : 3 * MT] = b_rz[H : 2 * H].reshape(MT, 128).T
    bias_pack[:, 3 * MT : 4 * MT] = b_hh[2 * H :].reshape(MT, 128).T
    bias_pack[:, 4 * MT : 5 * MT] = b_ih[2 * H :].reshape(MT, 128).T
    bias_pack[0:2, 5 * MT] = b_out

    shared = {
        "whhPack": whh_pack,
        "w_ihT": w_ih_pad,
        "woutKM": wout_pad,
        "biasPack": bias_pack,
    }
    return shared


def kernel(
    ev_context,
    teacher_deltas,
    w_init,
    b_init,
    w_ih,
    w_hh,
    b_ih,
    b_hh,
    w_out,
    b_out,
    steps=T,
    trace=False,
):
    from concourse.bass_utils import run_bass_kernel_spmd

    ev_context = np.asarray(ev_context, np.float32)
    teacher_deltas = np.asarray(teacher_deltas, np.float32)

    nc = _get_program(steps)
    shared = _shared_inputs(
        np.asarray(w_init, np.float32),
        np.asarray(b_init, np.float32),
        np.asarray(w_ih, np.float32),
        np.asarray(w_hh, np.float32),
        np.asarray(b_ih, np.float32),
        np.asarray(b_hh, np.float32),
        np.asarray(w_out, np.float32),
        np.asarray(b_out, np.float32),
    )
    winitT16 = np.ascontiguousarray(np.asarray(w_init, np.float32).T).astype(np.float16)
    in_maps = []
    for c in range(NCORES):
        sl = slice(c * BS, (c + 1) * BS)
        in_maps.append(
            _prep_core_inputs(
                ev_context[sl], teacher_deltas[sl, :steps], shared, winitT16
            )
        )

    res = run_bass_kernel_spmd(nc, in_maps, list(range(NCORES)), trace=trace)

    pred_deltas = np.empty((B, steps, 2), np.float32)
    pred_pos = np.empty((B, steps, 2), np.float32)
    for c in range(NCORES):
        sl = slice(c * BS, (c + 1) * BS)
        pred_deltas[sl] = res.results[c]["deltasT"].transpose(2, 0, 1)
        pred_pos[sl] = res.results[c]["posT"].transpose(2, 0, 1)
    if trace:
        return (pred_deltas, pred_pos), res
    return pred_deltas, pred_pos
